# revision 13
# baseline (speedup 1.0000x reference)
"""GQA attention (B=2,S=2048,D=2048,H=32,Hkv=8,dh=64) + RoPE + causal + out-proj
on 8 Trainium2 NeuronCores.

Sharding: data-parallel over batch (2) x tensor-parallel over KV-head pairs (4).
Core c handles batch b=c//4 and KV heads {2g, 2g+1} with g=c%4, plus their
8 Q heads. Wo is row-sharded, so each core emits a partial [S, D] output;
the host sums the 4 partials per batch. RoPE'd K and V head slices are
emitted directly.

All on-device matmuls run in float32r (fp32 storage, full PE rate, ~1e-4 rel
err). The kernel works in transposed layouts throughout: x arrives as x^T,
projections produce Q^T/K^T/V^T (features on partitions), scores are computed
as S^T[t, s] so softmax'd probabilities are already in the right layout for
the A@V matmul, whose output attn^T[feat, s] is directly the stationary
operand of the Wo projection. Softmax denominators come from a ones-column
appended to V. Causal structure: fully-masked blocks are skipped, diagonal
blocks restrict the matmul to valid columns plus a 128x128 additive triangle
mask.
"""

import numpy as np
from contextlib import ExitStack

import concourse.bass as bass
import concourse.tile as tile
from concourse import bacc, mybir
from concourse import bass_utils
from concourse.alu_op_type import AluOpType

F32R = mybir.dt.float32r
FP32 = mybir.dt.float32
AFT = mybir.ActivationFunctionType

B, S, D = 2, 2048, 2048
N_HEADS, N_KV, HD = 32, 8, 64
N_REP = N_HEADS // N_KV
ROPE_BASE = 10000.0
NCORES = 8

QF = 512          # q features per core (8 heads x 64)
KVF = 128         # k (or v) features per core (2 heads x 64)
KC = D // 128     # contraction chunks for projections (16)
NS = S // 512     # 512-wide s blocks (4)
NSC = S // 128    # 128-wide s chunks (16)
NT = S // 128     # t blocks (16)
ND = D // 512     # 512-wide d blocks (4)
NEG = -1.0e30


def _build_program():
    nc = bacc.Bacc("TRN2", target_bir_lowering=False, debug=False,
                   num_devices=NCORES)
    xT = nc.dram_tensor("xT", [D, S], F32R, kind="ExternalInput").ap()
    wq = nc.dram_tensor("wq", [D, QF], F32R, kind="ExternalInput").ap()
    wkv = nc.dram_tensor("wkv", [D, 2 * KVF], F32R, kind="ExternalInput").ap()
    wo = nc.dram_tensor("wo", [QF, D], F32R, kind="ExternalInput").ap()
    cosd = nc.dram_tensor("cosd", [128, S], F32R, kind="ExternalInput").ap()
    sins = nc.dram_tensor("sins", [128, S], F32R, kind="ExternalInput").ap()
    trimask = nc.dram_tensor("trimask", [128, 128], FP32, kind="ExternalInput").ap()
    identd = nc.dram_tensor("identd", [128, 64], F32R, kind="ExternalInput").ap()
    onesc = nc.dram_tensor("onesc", [128, NT], F32R, kind="ExternalInput").ap()
    po = nc.dram_tensor("po", [S, D], FP32, kind="ExternalOutput").ap()
    ko = nc.dram_tensor("ko", [KVF, S], F32R, kind="ExternalOutput").ap()
    vo = nc.dram_tensor("vo", [KVF, S], F32R, kind="ExternalOutput").ap()

    with tile.TileContext(nc) as tc, \
            nc.allow_low_precision(reason="float32r matmul pipeline"), \
            ExitStack() as ctx:
        _kernel(ctx, tc, nc, xT=xT, wq=wq, wkv=wkv, wo=wo, cosd=cosd,
                sins=sins, trimask=trimask, identd=identd, onesc=onesc,
                po=po, ko=ko, vo=vo)
    nc.compile()
    return nc


def _kernel(ctx, tc, nc, *, xT, wq, wkv, wo, cosd, sins, trimask, identd,
            onesc, po, ko, vo):
    const = ctx.enter_context(tc.tile_pool(name="const", bufs=1))
    cos_t = const.tile([128, S], F32R)
    sin_t = const.tile([128, S], F32R)
    tri_t = const.tile([128, 128], FP32)
    id_t = const.tile([128, 64], F32R)
    nc.sync.dma_start(cos_t[:], cosd[:])
    nc.sync.dma_start(sin_t[:], sins[:])
    nc.sync.dma_start(tri_t[:], trimask[:])
    nc.sync.dma_start(id_t[:], identd[:])

    persist = ctx.enter_context(tc.tile_pool(name="persist", bufs=1))
    # q tiles: tile m holds q heads 2m (parts 0:64) and 2m+1 (parts 64:128)
    qt = [persist.tile([128, S], F32R, tag=f"qt{m}", name=f"qt{m}") for m in range(4)]
    # ktr_dup[g]: roped K head g duplicated on both partition halves
    ktr = [persist.tile([128, S], F32R, tag=f"ktr{g}", name=f"ktr{g}") for g in range(2)]

    kv_pool = ctx.enter_context(tc.tile_pool(name="kv", bufs=1))
    kvt1 = kv_pool.tile([128, S], F32R, tag="kvt1", name="kvt1")

    # ---- Phase 1: projections (Q/K/V), with RoPE fused per 512-col slice ----
    with tc.tile_pool(name="wproj", bufs=1) as wpool, \
            tc.tile_pool(name="xin", bufs=4) as xpool, \
            tc.tile_pool(name="ropes", bufs=2) as rpool, \
            tc.tile_pool(name="pproj", bufs=1, space="PSUM") as ppsum:
        wq_t = wpool.tile([128, KC, QF], F32R)
        wkv_t = wpool.tile([128, KC, 2 * KVF], F32R)
        nc.sync.dma_start(wq_t[:], wq.rearrange("(c p) f -> p c f", p=128))
        nc.sync.dma_start(wkv_t[:], wkv.rearrange("(c p) f -> p c f", p=128))

        for n in range(NS):            # 512-wide s block
            s0 = 512 * n
            pss = [ppsum.tile([128, 512], FP32, tag=f"proj{m}", name=f"proj{m}")
                   for m in range(6)]
            for k in range(KC):
                xtile = xpool.tile([128, 512], F32R, tag="xt", name="xt")
                nc.sync.dma_start(xtile[:], xT[128 * k:128 * (k + 1), s0:s0 + 512])
                for m in range(6):
                    if m < 4:
                        lhs = wq_t[:, k, 128 * m:128 * (m + 1)]
                    else:
                        lhs = wkv_t[:, k, 128 * (m - 4):128 * (m - 3)]
                    nc.tensor.matmul(pss[m][:], lhs, xtile[:],
                                     start=(k == 0), stop=(k == KC - 1))
            for m in range(6):         # 4 q chunks + 2 kv chunks
                ps = pss[m]
                if m == 5:             # V^T: plain copy, no rope
                    nc.vector.tensor_copy(kvt1[:, s0:s0 + 512], ps[:])
                    continue
                dst = qt[m] if m < 4 else None
                # RoPE on this [128, 512] slice into dst.
                # tmp = swap32(q) * sinswap ; q2 = q * cos ; dst = tmp + q2
                tmp = rpool.tile([128, 512], F32R, tag="rtmp")
                q2 = rpool.tile([128, 512], F32R, tag="rq2")
                for o in (0, 64):
                    nc.vector.tensor_tensor(
                        tmp[o:o + 32, :], ps[o + 32:o + 64, :],
                        sin_t[o + 32:o + 64, s0:s0 + 512], AluOpType.mult)
                    nc.vector.tensor_tensor(
                        tmp[o + 32:o + 64, :], ps[o:o + 32, :],
                        sin_t[o:o + 32, s0:s0 + 512], AluOpType.mult)
                nc.vector.tensor_tensor(q2[:], ps[:], cos_t[:, s0:s0 + 512],
                                        AluOpType.mult)
                if m < 4:
                    nc.vector.tensor_tensor(dst[:, s0:s0 + 512], tmp[:], q2[:],
                                            AluOpType.add)
                else:
                    # K^T: write each head's rope result to both halves
                    for g in range(2):
                        b0 = 64 * g
                        for o in (0, 64):
                            nc.vector.tensor_tensor(
                                ktr[g][o:o + 64, s0:s0 + 512],
                                tmp[b0:b0 + 64, :], q2[b0:b0 + 64, :],
                                AluOpType.add)

    # K / V outputs (head g at rows 64g:64g+64)
    for g in range(2):
        nc.sync.dma_start(ko[64 * g:64 * (g + 1), :], ktr[g][0:64, :])
    nc.sync.dma_start(vo[:], kvt1[:])

    persist2 = ctx.enter_context(tc.tile_pool(name="persist2", bufs=1))
    # V' tiles: [t within block, t-block, 64 v feats + ones]
    vt = [persist2.tile([128, NT, HD + 1], F32R, tag=f"vt{g}", name=f"vt{g}") for g in range(2)]
    # attn^T tiles: tile fc holds feats of heads 2fc, 2fc+1
    at = [persist2.tile([128, S], F32R, tag=f"at{fc}", name=f"at{fc}") for fc in range(4)]

    # ---- Phase 2: V transpose -> V' tiles (with ones column) ----
    with tc.tile_pool(name="ptr", bufs=2, space="PSUM") as tpsum:
        for g in range(2):
            nc.sync.dma_start(vt[g][:, :, HD:HD + 1], onesc[:])
            for tb in range(NT):
                tps = tpsum.tile([128, 64], F32R, tag="tps")
                nc.tensor.transpose(tps[:], kvt1[64 * g:64 * (g + 1),
                                                 128 * tb:128 * (tb + 1)],
                                    id_t[64 * g:64 * (g + 1), :])
                nc.vector.tensor_copy(vt[g][:, tb, 0:HD], tps[:])

    # ---- Phase 3: attention per head pair ----
    spool = ctx.enter_context(tc.tile_pool(name="expst", bufs=6))
    bcpool = ctx.enter_context(tc.tile_pool(name="bc", bufs=2))
    spsum = ctx.enter_context(tc.tile_pool(name="psc", bufs=4, space="PSUM"))
    opsum = ctx.enter_context(tc.tile_pool(name="pav", bufs=2, space="PSUM"))

    for hp in range(4):                 # head pair (tile), heads 2hp, 2hp+1
        g2 = hp // 2                    # local kv head
        for sb in range(NS):
            s0 = 512 * sb
            n_t = 4 * (sb + 1)
            ot = [opsum.tile([HD + 1, 512], FP32, tag="ot", name="ot") for _ in range(2)]
            for tb in range(n_t):
                t0 = 128 * tb
                off = max(0, t0 - s0)
                sps = [spsum.tile([128, 512], FP32, tag="sps", name="sps") for _ in range(2)]
                ex = [spool.tile([128, 512], F32R, tag="ex", name="ex") for _ in range(2)]
                for j in range(2):      # head j of the pair
                    b0 = 64 * j
                    nc.tensor.matmul(
                        sps[j][:, off:], ktr[g2][b0:b0 + 64, t0:t0 + 128],
                        qt[hp][b0:b0 + 64, s0 + off:s0 + 512],
                        start=True, stop=True, tile_position=(b0, 0))
                    if t0 >= s0:        # diagonal block: triangle mask
                        nc.vector.tensor_tensor(
                            sps[j][:, off:off + 128], sps[j][:, off:off + 128],
                            tri_t[:], AluOpType.add)
                    nc.scalar.activation(ex[j][:, off:], sps[j][:, off:], AFT.Exp)
                    nc.tensor.matmul(
                        ot[j][:, off:], vt[g2][:, tb, :], ex[j][:, off:],
                        start=(tb == 0), stop=(tb == n_t - 1))
            for j in range(2):
                rec = bcpool.tile([1, 512], FP32, tag="rec", name="rec")
                nc.vector.reciprocal(rec[:], ot[j][HD:HD + 1, :])
                bc = bcpool.tile([64, 512], FP32, tag="bc", name="bc")
                nc.gpsimd.partition_broadcast(bc[:], rec[:])
                nc.vector.tensor_tensor(at[hp][64 * j:64 * (j + 1), s0:s0 + 512],
                                        ot[j][0:HD, :], bc[:], AluOpType.mult)

    # ---- Phase 4: output projection (partial over this core's features) ----
    with tc.tile_pool(name="wout", bufs=1) as wopool, \
            tc.tile_pool(name="oout", bufs=3) as outpool, \
            tc.tile_pool(name="pwo", bufs=2, space="PSUM") as wpsum:
        wo_t = wopool.tile([128, 4, D], F32R)
        nc.sync.dma_start(wo_t[:], wo.rearrange("(c p) f -> p c f", p=128))
        for sc in range(NSC):
            s1 = 128 * sc
            for dc in range(ND):
                d0 = 512 * dc
                pps = wpsum.tile([128, 512], FP32, tag="pps")
                for fc in range(4):
                    nc.tensor.matmul(pps[:], at[fc][:, s1:s1 + 128],
                                     wo_t[:, fc, d0:d0 + 512],
                                     start=(fc == 0), stop=(fc == 3))
                outs = outpool.tile([128, 512], FP32, tag="outs")
                nc.scalar.copy(outs[:], pps[:])
                nc.sync.dma_start(po[s1:s1 + 128, d0:d0 + 512], outs[:])


_NC_CACHE = None


def _get_program():
    global _NC_CACHE
    if _NC_CACHE is None:
        _NC_CACHE = _build_program()
    return _NC_CACHE


def _host_inputs(x, cos, sin, Wq, Wk, Wv, Wo):
    """Build the 8 per-core input maps."""
    x = np.asarray(x, np.float32)
    cos = np.asarray(cos, np.float32)
    sin = np.asarray(sin, np.float32)
    Wq = np.asarray(Wq, np.float32)
    Wk = np.asarray(Wk, np.float32)
    Wv = np.asarray(Wv, np.float32)
    Wo = np.asarray(Wo, np.float32)

    cosT = np.ascontiguousarray(cos.T)                       # [64, S]
    cosd = np.concatenate([cosT, cosT], axis=0)              # [128, S]
    sinT = np.ascontiguousarray(sin.T)                       # [64, S]
    sinswap = np.concatenate([sinT[32:], -sinT[:32]], axis=0)  # [64, S]
    sins = np.concatenate([sinswap, sinswap], axis=0)        # [128, S]

    ti, sj = np.meshgrid(np.arange(128), np.arange(128), indexing="ij")
    trimask = np.where(ti <= sj, 0.0, NEG).astype(np.float32)
    ident = np.eye(64, dtype=np.float32)
    identd = np.concatenate([ident, ident], axis=0)          # [128, 64]

    scale = 1.0 / np.sqrt(np.float32(HD))
    in_maps = []
    for c in range(NCORES):
        b, g = divmod(c, 4)
        in_maps.append({
            "xT": np.ascontiguousarray(x[b].T),
            "wq": np.ascontiguousarray(Wq[:, 512 * g:512 * (g + 1)] * scale),
            "wkv": np.ascontiguousarray(np.concatenate(
                [Wk[:, 128 * g:128 * (g + 1)],
                 Wv[:, 128 * g:128 * (g + 1)]], axis=1)),
            "wo": np.ascontiguousarray(Wo[512 * g:512 * (g + 1), :]),
            "cosd": cosd, "sins": sins, "trimask": trimask, "identd": identd,
            "onesc": np.ones((128, NT), np.float32),
        })
    return in_maps


def _assemble(results):
    out = np.zeros((B, S, D), np.float32)
    k_full = np.zeros((B, N_KV, S, HD), np.float32)
    v_full = np.zeros((B, N_KV, S, HD), np.float32)
    for c in range(NCORES):
        b, g = divmod(c, 4)
        r = results[c]
        out[b] += r["po"]
        for j in range(2):
            k_full[b, 2 * g + j] = r["ko"][64 * j:64 * (j + 1), :].T
            v_full[b, 2 * g + j] = r["vo"][64 * j:64 * (j + 1), :].T
    return out, k_full, v_full


def kernel(x, cos, sin, Wq, Wk, Wv, Wo):
    nc = _get_program()
    in_maps = _host_inputs(x, cos, sin, Wq, Wk, Wv, Wo)
    res = bass_utils.run_bass_kernel_spmd(nc, in_maps,
                                          core_ids=list(range(NCORES)))
    return _assemble(res.results)


# revision 17
# speedup vs baseline: 7.9916x; 7.9916x over previous
"""GQA attention (B=2,S=2048,D=2048,H=32,Hkv=8,dh=64) + RoPE + causal + out-proj
on 8 Trainium2 NeuronCores.

Sharding: data-parallel over batch (2) x tensor-parallel over KV-head pairs (4).
Core c handles batch b=c//4 and KV heads {2g, 2g+1} with g=c%4, plus their
8 Q heads. Wo is row-sharded, so each core emits a partial [S, D] output;
the host sums the 4 partials per batch. RoPE'd K and V head slices are
emitted directly.

All on-device matmuls run in float32r (fp32 storage, full PE rate, ~1e-4 rel
err). The kernel works in transposed layouts throughout: x arrives as x^T,
projections produce Q^T/K^T/V^T (features on partitions), scores are computed
as S^T[t, s] so softmax'd probabilities are already in the right layout for
the A@V matmul, whose output attn^T[feat, s] is directly the stationary
operand of the Wo projection. Softmax denominators come from a ones-column
appended to V. Causal structure: fully-masked blocks are skipped, diagonal
blocks restrict the matmul to valid columns plus a 128x128 additive triangle
mask.

Pipelining (engines execute their instruction streams in order, so emission
order is execution order):
 - per 512-wide s-block: projection in two 3-chunk passes (x re-streamed
   twice), with the previous block's output-projection pieces interleaved
   into the k-loops; fused per-slice RoPE; V transposes; then attention.
 - the attention inner loop is software-pipelined: A@V for block tb issues
   after the scores for tb+1, so TensorE never waits on ScalarE's exp.
PSUM budget (8 banks): proj acc 3 + wo 1 + scores 2 + attn-out 2.
"""

import numpy as np
from contextlib import ExitStack

import concourse.bass as bass
import concourse.tile as tile
from concourse import bacc, mybir
from concourse import bass_utils
from concourse.alu_op_type import AluOpType

F32R = mybir.dt.float32r
FP32 = mybir.dt.float32
AFT = mybir.ActivationFunctionType

B, S, D = 2, 2048, 2048
N_HEADS, N_KV, HD = 32, 8, 64
NCORES = 8

QF = 512          # q features per core (8 heads x 64)
KVF = 128         # k (or v) features per core (2 heads x 64)
KC = D // 128     # contraction chunks for projections (16)
NS = S // 512     # 512-wide s blocks (4)
NT = S // 128     # t blocks (16)
ND = D // 512     # 512-wide d blocks (4)
NEG = -1.0e30


def _build_program():
    nc = bacc.Bacc("TRN2", target_bir_lowering=False, debug=False,
                   num_devices=NCORES)
    xT = nc.dram_tensor("xT", [D, S], F32R, kind="ExternalInput").ap()
    wq = nc.dram_tensor("wq", [D, QF], F32R, kind="ExternalInput").ap()
    wkv = nc.dram_tensor("wkv", [D, 2 * KVF], F32R, kind="ExternalInput").ap()
    wo = nc.dram_tensor("wo", [QF, D], F32R, kind="ExternalInput").ap()
    cosd = nc.dram_tensor("cosd", [128, S], F32R, kind="ExternalInput").ap()
    sins = nc.dram_tensor("sins", [128, S], F32R, kind="ExternalInput").ap()
    trimask = nc.dram_tensor("trimask", [128, 128], FP32, kind="ExternalInput").ap()
    identd = nc.dram_tensor("identd", [128, 64], F32R, kind="ExternalInput").ap()
    onesc = nc.dram_tensor("onesc", [128, NT], F32R, kind="ExternalInput").ap()
    po = nc.dram_tensor("po", [S, D], FP32, kind="ExternalOutput").ap()
    ko = nc.dram_tensor("ko", [KVF, S], F32R, kind="ExternalOutput").ap()
    vo = nc.dram_tensor("vo", [KVF, S], F32R, kind="ExternalOutput").ap()

    with tile.TileContext(nc) as tc, \
            nc.allow_low_precision(reason="float32r matmul pipeline"), \
            ExitStack() as ctx:
        _kernel(ctx, tc, nc, xT=xT, wq=wq, wkv=wkv, wo=wo, cosd=cosd,
                sins=sins, trimask=trimask, identd=identd, onesc=onesc,
                po=po, ko=ko, vo=vo)
    nc.compile()
    return nc


def _kernel(ctx, tc, nc, *, xT, wq, wkv, wo, cosd, sins, trimask, identd,
            onesc, po, ko, vo):
    const = ctx.enter_context(tc.tile_pool(name="const", bufs=1))
    tri_t = const.tile([128, 128], FP32)
    id_t = const.tile([128, 64], F32R)

    persist = ctx.enter_context(tc.tile_pool(name="persist", bufs=1))
    # roped K head g, duplicated on both partition halves (for row tiling)
    ktr = [persist.tile([128, S], F32R, tag=f"ktr{g}", name=f"ktr{g}")
           for g in range(2)]
    # V' tiles: [t within block, t-block, 64 v feats + ones col]
    vt = [persist.tile([128, NT, HD + 1], F32R, tag=f"vt{g}", name=f"vt{g}")
          for g in range(2)]

    # PSUM pools, 8 banks: proj acc 3 + wo 1 + scores 2 + attn-out 2
    accp = ctx.enter_context(tc.tile_pool(name="accp", bufs=3, space="PSUM"))
    wop = ctx.enter_context(tc.tile_pool(name="wop", bufs=1, space="PSUM"))
    spsp = ctx.enter_context(tc.tile_pool(name="spsp", bufs=2, space="PSUM"))
    otp = ctx.enter_context(tc.tile_pool(name="otp", bufs=1, space="PSUM"))

    with tc.tile_pool(name="wproj", bufs=1) as wpool, \
            tc.tile_pool(name="tabs", bufs=1) as tabpool, \
            tc.tile_pool(name="xin", bufs=4) as xpool, \
            tc.tile_pool(name="stage", bufs=2) as stpool, \
            tc.tile_pool(name="ropes", bufs=2) as rpool, \
            tc.tile_pool(name="qsl", bufs=2) as qpool, \
            tc.tile_pool(name="kvsl", bufs=1) as kvpool, \
            tc.tile_pool(name="expool", bufs=4) as expool, \
            tc.tile_pool(name="bcp", bufs=1) as bcpool, \
            tc.tile_pool(name="asl", bufs=2) as aspool, \
            tc.tile_pool(name="oout", bufs=2) as outpool:
        # weights first in the DMA queues so the first pass starts early
        wq_t = wpool.tile([128, KC, QF], F32R)
        wkv_t = wpool.tile([128, KC, 2 * KVF], F32R)
        for m in range(4):
            nc.sync.dma_start(
                wq_t[:, :, 128 * m:128 * (m + 1)],
                wq[:, 128 * m:128 * (m + 1)].rearrange("(c p) f -> p c f", p=128))
        nc.sync.dma_start(wkv_t[:], wkv.rearrange("(c p) f -> p c f", p=128))
        cos_t = tabpool.tile([128, S], F32R)
        sin_t = tabpool.tile([128, S], F32R)
        nc.sync.dma_start(cos_t[:], cosd[:])
        nc.sync.dma_start(sin_t[:], sins[:])
        nc.sync.dma_start(tri_t[:], trimask[:])
        nc.sync.dma_start(id_t[:], identd[:])
        for g in range(2):
            nc.sync.dma_start(vt[g][:, :, HD:HD + 1], onesc[:])
        wo_t = wpool.tile([128, 4, D], F32R)
        nc.sync.dma_start(wo_t[:], wo.rearrange("(c p) f -> p c f", p=128))

        qs = {}
        asl = {}

        def emit_wo_piece(n_src, piece, pool, tag):
            """One output-projection piece: s-chunk sc, d-block dc of block
            n_src (4 fc-matmuls + copy + store)."""
            sc, dc = divmod(piece, ND)
            s1 = 128 * sc
            d0 = 512 * dc
            pps = pool.tile([128, 512], FP32, tag=tag, name="pps")
            for fc in range(4):
                nc.tensor.matmul(pps[:], asl[(fc, n_src)][:, s1:s1 + 128],
                                 wo_t[:, fc, d0:d0 + 512],
                                 start=(fc == 0), stop=(fc == 3))
            outs = outpool.tile([128, 512], FP32, tag="outs", name="outs")
            nc.vector.tensor_copy(outs[:], pps[:])
            nc.sync.dma_start(
                po[512 * n_src + s1:512 * n_src + s1 + 128, d0:d0 + 512],
                outs[:])

        def rope_slice(ps_or_st, m, s0):
            """RoPE a [128, 512] staged projection slice into its target."""
            st = ps_or_st
            tmp = rpool.tile([128, 512], F32R, tag="rsc", name="tmp")
            q2 = rpool.tile([128, 512], F32R, tag="rsc", name="q2")
            for o in (0, 64):
                nc.vector.tensor_tensor(
                    tmp[o:o + 32, :], st[o + 32:o + 64, :],
                    sin_t[o + 32:o + 64, s0:s0 + 512], AluOpType.mult)
                nc.vector.tensor_tensor(
                    tmp[o + 32:o + 64, :], st[o:o + 32, :],
                    sin_t[o:o + 32, s0:s0 + 512], AluOpType.mult)
            nc.vector.tensor_tensor(q2[:], st[:], cos_t[:, s0:s0 + 512],
                                    AluOpType.mult)
            if m < 4:
                qsl = qpool.tile([128, 512], F32R, tag=f"qs{m}", name=f"qs{m}")
                nc.vector.tensor_tensor(qsl[:], tmp[:], q2[:], AluOpType.add)
                return qsl
            for g in range(2):
                b0 = 64 * g
                for o in (0, 64):
                    nc.vector.tensor_tensor(
                        ktr[g][o:o + 64, s0:s0 + 512],
                        tmp[b0:b0 + 64, :], q2[b0:b0 + 64, :], AluOpType.add)
            return None

        for n in range(NS):
            s0 = 512 * n
            # previous block's output-projection pieces, interleaved below
            wo_pieces = iter(range(16)) if n > 0 else iter(())

            # ---- projection: two 3-chunk passes (x re-streamed per pass) ---
            for pi, mtrip in enumerate(((0, 1, 2), (3, 4, 5))):
                pms = [accp.tile([128, 512], FP32, tag="acc", name=f"pj{m}")
                       for m in mtrip]
                for k in range(KC):
                    xt = xpool.tile([128, 512], F32R, tag="xt", name="xt")
                    nc.sync.dma_start(
                        xt[:], xT[128 * k:128 * (k + 1), s0:s0 + 512])
                    for m, pm in zip(mtrip, pms):
                        if m < 4:
                            lhs = wq_t[:, k, 128 * m:128 * (m + 1)]
                        else:
                            lhs = wkv_t[:, k, 128 * (m - 4):128 * (m - 3)]
                        nc.tensor.matmul(pm[:], lhs, xt[:],
                                         start=(k == 0), stop=(k == KC - 1))
                    if k % 2 == 1:
                        p = next(wo_pieces, None)
                        if p is not None:
                            emit_wo_piece(n - 1, p, wop, "wot")
                for m, pm in zip(mtrip, pms):
                    if m == 5:
                        kvs = kvpool.tile([128, 512], F32R, tag="kvs",
                                          name="kvs")
                        nc.scalar.copy(kvs[:], pm[:])
                        nc.sync.dma_start(vo[:, s0:s0 + 512], kvs[:])
                        for g in range(2):
                            for i in range(4):
                                tb = 4 * n + i
                                tp = spsp.tile([128, 64], F32R, tag="sps",
                                               name="tp")
                                nc.tensor.transpose(
                                    tp[:], kvs[64 * g:64 * (g + 1),
                                               128 * i:128 * (i + 1)],
                                    id_t[64 * g:64 * (g + 1), :])
                                nc.vector.tensor_copy(vt[g][:, tb, 0:HD],
                                                      tp[:])
                        continue
                    st = stpool.tile([128, 512], F32R, tag="stage", name="st")
                    nc.scalar.copy(st[:], pm[:])
                    q = rope_slice(st, m, s0)
                    if q is not None:
                        qs[(m, n)] = q

            # any leftover wo pieces for block n-1
            for p in wo_pieces:
                emit_wo_piece(n - 1, p, wop, "wot")

            # ---- attention for s-block n (software-pipelined inner loop) ---
            n_t = 4 * (n + 1)
            for hp in range(4):
                g2 = hp // 2
                qsl = qs[(hp, n)]
                ot = otp.tile([HD + 1, 1024], FP32, tag="ot", name="ot")
                pend = None     # (ex, off) waiting for its A@V
                for tb in range(n_t):
                    t0 = 128 * tb
                    off = max(0, t0 - s0)
                    ex = expool.tile([128, 1024], F32R, tag="ex", name="ex")
                    for j in range(2):
                        b0 = 64 * j
                        sp = spsp.tile([128, 512], FP32, tag="sps", name="sp")
                        nc.tensor.matmul(
                            sp[:, off:], ktr[g2][b0:b0 + 64, t0:t0 + 128],
                            qsl[b0:b0 + 64, off:512],
                            start=True, stop=True, tile_position=(b0, 0))
                        if t0 >= s0:    # diagonal: triangle mask
                            nc.vector.tensor_tensor(
                                sp[:, off:off + 128], sp[:, off:off + 128],
                                tri_t[:], AluOpType.add)
                        nc.scalar.activation(ex[:, 512 * j + off:512 * (j + 1)],
                                             sp[:, off:], AFT.Exp)
                    if pend is not None:
                        _emit_av(nc, ot, vt[g2], *pend, n_t)
                    pend = (ex, tb, off)
                _emit_av(nc, ot, vt[g2], *pend, n_t)
                a_t = aspool.tile([128, 512], F32R, tag=f"as{hp}",
                                  name=f"as{hp}")
                asl[(hp, n)] = a_t
                for j in range(2):
                    rec = bcpool.tile([1, 512], FP32, tag="rec", name="rec")
                    nc.vector.reciprocal(rec[:], ot[HD:HD + 1,
                                                    512 * j:512 * (j + 1)])
                    bc = bcpool.tile([64, 512], FP32, tag="bc", name="bc")
                    nc.gpsimd.partition_broadcast(bc[:], rec[:])
                    nc.vector.tensor_tensor(
                        a_t[64 * j:64 * (j + 1), :],
                        ot[0:HD, 512 * j:512 * (j + 1)], bc[:],
                        AluOpType.mult)

        # tail: output projection for the last s-block (acc pool is idle now)
        for p in range(16):
            emit_wo_piece(NS - 1, p, accp, "acc")

    # K output (head g at rows 64g:64g+64); V was written per s-block
    for g in range(2):
        nc.sync.dma_start(ko[64 * g:64 * (g + 1), :], ktr[g][0:64, :])


def _emit_av(nc, ot, vtg, ex, tb, off, n_t):
    for j in range(2):
        c0 = 512 * j + off
        nc.tensor.matmul(ot[:, c0:512 * (j + 1)], vtg[:, tb, :],
                         ex[:, c0:512 * (j + 1)],
                         start=(tb == 0), stop=(tb == n_t - 1))


_NC_CACHE = None


def _get_program():
    global _NC_CACHE
    if _NC_CACHE is None:
        _NC_CACHE = _build_program()
    return _NC_CACHE


def _host_inputs(x, cos, sin, Wq, Wk, Wv, Wo):
    """Build the 8 per-core input maps."""
    x = np.asarray(x, np.float32)
    cos = np.asarray(cos, np.float32)
    sin = np.asarray(sin, np.float32)
    Wq = np.asarray(Wq, np.float32)
    Wk = np.asarray(Wk, np.float32)
    Wv = np.asarray(Wv, np.float32)
    Wo = np.asarray(Wo, np.float32)

    cosT = np.ascontiguousarray(cos.T)                       # [64, S]
    cosd = np.concatenate([cosT, cosT], axis=0)              # [128, S]
    sinT = np.ascontiguousarray(sin.T)                       # [64, S]
    sinswap = np.concatenate([sinT[32:], -sinT[:32]], axis=0)  # [64, S]
    sins = np.concatenate([sinswap, sinswap], axis=0)        # [128, S]

    ti, sj = np.meshgrid(np.arange(128), np.arange(128), indexing="ij")
    trimask = np.where(ti <= sj, 0.0, NEG).astype(np.float32)
    ident = np.eye(64, dtype=np.float32)
    identd = np.concatenate([ident, ident], axis=0)          # [128, 64]

    scale = 1.0 / np.sqrt(np.float32(HD))
    in_maps = []
    for c in range(NCORES):
        b, g = divmod(c, 4)
        in_maps.append({
            "xT": np.ascontiguousarray(x[b].T),
            "wq": np.ascontiguousarray(Wq[:, 512 * g:512 * (g + 1)] * scale),
            "wkv": np.ascontiguousarray(np.concatenate(
                [Wk[:, 128 * g:128 * (g + 1)],
                 Wv[:, 128 * g:128 * (g + 1)]], axis=1)),
            "wo": np.ascontiguousarray(Wo[512 * g:512 * (g + 1), :]),
            "cosd": cosd, "sins": sins, "trimask": trimask, "identd": identd,
            "onesc": np.ones((128, NT), np.float32),
        })
    return in_maps


def _assemble(results):
    out = np.zeros((B, S, D), np.float32)
    k_full = np.zeros((B, N_KV, S, HD), np.float32)
    v_full = np.zeros((B, N_KV, S, HD), np.float32)
    for c in range(NCORES):
        b, g = divmod(c, 4)
        r = results[c]
        out[b] += r["po"]
        for j in range(2):
            k_full[b, 2 * g + j] = r["ko"][64 * j:64 * (j + 1), :].T
            v_full[b, 2 * g + j] = r["vo"][64 * j:64 * (j + 1), :].T
    return out, k_full, v_full


def kernel(x, cos, sin, Wq, Wk, Wv, Wo):
    nc = _get_program()
    in_maps = _host_inputs(x, cos, sin, Wq, Wk, Wv, Wo)
    res = bass_utils.run_bass_kernel_spmd(nc, in_maps,
                                          core_ids=list(range(NCORES)))
    return _assemble(res.results)
